# revision 37
# baseline (speedup 1.0000x reference)
"""GCN+JumpingKnowledge distributed Trainium2 kernel (8 NeuronCores).

Strategy: shard destination nodes across 8 cores (6250 each). Per layer:
  - sharded feature transform z^T = W @ act^T on TensorE; transpose to
    row-major into persistent SBUF z_rows, DMA to HBM shard halves,
    A/B-split AllGather into Shared z_fullA/z_fullB fp16 (AG-A fires
    while B rows are still being produced; gathers sourced from A
    overlap AG-B's transfer; rebased indices keep both halves < 2^15
    for int16 gather indices)
  - dma_gather source rows for this core's edges (sorted by dst tile,
    split by A/B half, padded to a common per-(tile,half) block count
    across cores so one SPMD program fits all); gather calls
    round-robin over 4 SWDGE queues (desc-gen overlaps ~2x)
  - segment-sum via TensorE: psum[feat,dst] += G_blk^T @ S_blk where
    S_blk[e,d] = (dst_e == d) * normval_e is precomputed on host and
    streamed from HBM (layer-invariant; the leading ~8MB is persisted
    in SBUF across layers); self-loops are a diagonal S block whose
    lhsT is the local z_rows (no gather)
  - BN stats via per-tile accumulators + 1KB AllReduce, chunked
    scale/shift/ReLU on ACT so z-production starts early; JK max and
    final projection chunked to overlap conv3's tail.
"""

import os
import sys

import numpy as np

sys.path.insert(0, "/opt/trn_rl_repo")

N = 50000
E = 800000
F = 128
OUTF = 64
N_CORES = 8
SHARD = N // N_CORES  # 6250
TILE = 128
NTILE = (SHARD + TILE - 1) // TILE  # 49
LAST_W = SHARD - (NTILE - 1) * TILE  # 106
ASPLIT = 2560  # rows-per-shard split for the A/B AllGather (5 z-chunks)
BSPLIT = SHARD - ASPLIT  # 3690
GRP = 7  # tiles per gather group
BN_EPS = 1e-5
ZCHUNK = 512
NQ = 4  # SWDGE queues
MAX_CALL = 1024  # hw limit: 1024 idxs per dma_gather call


def _preprocess(edge_index):
    """Host-side edge routing. Returns (structure, per_core_arrays)."""
    src = np.asarray(edge_index[0], dtype=np.int64)
    dst = np.asarray(edge_index[1], dtype=np.int64)

    deg = np.bincount(dst, minlength=N).astype(np.float64) + 1.0
    dinv = 1.0 / np.sqrt(deg)
    dinv2 = (dinv * dinv).astype(np.float32)  # self-loop weights

    normval = (dinv[src] * dinv[dst]).astype(np.float32)

    core = dst // SHARD
    tile_id = (dst % SHARD) // TILE
    src_core = src // SHARD
    src_row = src % SHARD
    half = (src_row >= ASPLIT).astype(np.int64)  # 0 -> z_fullA, 1 -> z_fullB
    gidx = np.where(
        half == 0, src_core * ASPLIT + src_row, src_core * BSPLIT + src_row - ASPLIT
    )
    dstoff = ((dst % SHARD) % TILE).astype(np.int64)

    # per (core, tile, half) counts, padded to the max across cores
    key = (core * NTILE + tile_id) * 2 + half
    counts = np.bincount(key, minlength=N_CORES * NTILE * 2).reshape(
        N_CORES, NTILE, 2
    )
    maxcnt = counts.max(axis=0)  # [NTILE, 2]
    pad_blocks = (maxcnt + TILE - 1) // TILE  # blocks per (tile, half)

    # slot layout: groups of GRP tiles; per group all lo slots then all hi.
    groups = []
    slot_start = np.zeros((NTILE, 2), dtype=np.int64)
    cursor = 0
    for g0 in range(0, NTILE, GRP):
        tiles = list(range(g0, min(g0 + GRP, NTILE)))
        ginfo = {"tiles": tiles}
        for h, nm in ((0, "lo"), (1, "hi")):
            run_slot0 = cursor
            tb = []
            for t in tiles:
                slot_start[t, h] = cursor
                tb.append((cursor, int(pad_blocks[t, h])))
                cursor += int(pad_blocks[t, h]) * TILE
            run_slots = cursor - run_slot0
            calls = []
            o = run_slot0
            while o < run_slot0 + run_slots:
                n = min(MAX_CALL, run_slot0 + run_slots - o)
                calls.append((o, n))
                o += n
            ginfo[nm] = {
                "slot0": run_slot0,
                "nslots": run_slots,
                "tile_blocks": tb,
                "calls": calls,
            }
        groups.append(ginfo)
    total_slots = cursor
    total_blocks = total_slots // TILE

    # S stream layout: tile-major [diag, lo blocks, hi blocks] per tile.
    sblock_start = np.zeros(NTILE, dtype=np.int64)
    nsb = 0
    for t in range(NTILE):
        sblock_start[t] = nsb
        nsb += 1 + int(pad_blocks[t, 0]) + int(pad_blocks[t, 1])
    # map global gather block -> S stream block
    gblk2sblk = np.zeros(total_blocks, dtype=np.int64)
    for t in range(NTILE):
        for h in (0, 1):
            s0 = slot_start[t, h] // TILE
            base = sblock_start[t] + 1 + (int(pad_blocks[t, 0]) if h else 0)
            for j in range(int(pad_blocks[t, h])):
                gblk2sblk[s0 + j] = base + j

    # per-core slot content + S matrices
    per_core = []
    for c in range(N_CORES):
        m = core == c
        e_t = tile_id[m]
        e_h = half[m]
        e_src = gidx[m]
        e_nv = normval[m]
        e_do = dstoff[m]
        order = np.lexsort((e_h, e_t))
        e_t, e_h = e_t[order], e_h[order]
        e_src, e_nv, e_do = e_src[order], e_nv[order], e_do[order]
        # rank within (t, h) group
        k = e_t * 2 + e_h
        cnt_c = np.bincount(k, minlength=NTILE * 2)
        grp_starts = np.concatenate([[0], np.cumsum(cnt_c)[:-1]])
        rank = np.arange(len(k)) - grp_starts[k]
        slots = slot_start[e_t, e_h] + rank

        idx_vals = np.zeros(total_slots, dtype=np.int16)
        idx_vals[slots] = e_src.astype(np.int16)

        # idx wrapped layout: slot i -> partition i%16 (replicated x8), col i//16
        idx_arr = np.zeros((128, total_slots // 16), dtype=np.int16)
        v16 = idx_vals.reshape(-1, 16).T  # [16, total/16]
        for g in range(8):
            idx_arr[16 * g : 16 * g + 16] = v16

        # S matrices: [nsb, 128 e, 128 d] fp16
        smat = np.zeros((nsb, TILE, TILE), dtype=np.float16)
        e_sblk = gblk2sblk[slots // TILE]
        smat[e_sblk, slots % TILE, e_do] = e_nv.astype(np.float16)
        # diagonal self-loop blocks
        node0 = c * SHARD
        dia = np.arange(TILE)
        for t in range(NTILE):
            tw = LAST_W if t == NTILE - 1 else TILE
            blk = smat[sblock_start[t]]
            blk[dia[:tw], dia[:tw]] = dinv2[node0 + t * TILE : node0 + t * TILE + tw]
        # -> [128 e, nsb * 128 d] for DMA-friendly streaming
        smat_flat = np.ascontiguousarray(
            np.transpose(smat, (1, 0, 2)).reshape(TILE, nsb * TILE)
        )
        per_core.append({"idx": idx_arr, "smat": smat_flat})

    structure = {
        "groups": groups,
        "total_slots": total_slots,
        "total_blocks": total_blocks,
        "nsb": nsb,
        "sblock_start": sblock_start,
        "pad_blocks": pad_blocks,
    }
    return structure, per_core


def _build(structure):
    import concourse.bacc as bacc
    import concourse.tile as tile
    from concourse import mybir
    import concourse.bass as bass

    fp32 = mybir.dt.float32
    fp16 = mybir.dt.float16
    i16 = mybir.dt.int16
    AF = mybir.ActivationFunctionType
    OP = mybir.AluOpType

    groups = structure["groups"]
    total_slots = structure["total_slots"]
    nsb = structure["nsb"]
    sblock_start = structure["sblock_start"]
    pad_blocks = structure["pad_blocks"]
    max_sb = 1 + int(pad_blocks.sum(axis=1).max())  # largest per-tile S chunk

    nc = bacc.Bacc(
        "TRN2", target_bir_lowering=False, num_devices=N_CORES, num_swdge_queues=NQ
    )

    # ---- I/O ----
    xT_in = nc.declare_dram_parameter("xT", [F, SHARD], fp16, isOutput=False)
    idx_in = nc.declare_dram_parameter("idx", [128, total_slots // 16], i16, isOutput=False)
    smat_in = nc.declare_dram_parameter("smat", [128, nsb * TILE], fp16, isOutput=False)
    w_in = [
        nc.declare_dram_parameter(f"W{i}", [F, F], fp16, isOutput=False)
        for i in (1, 2, 3)
    ]
    wp_in = nc.declare_dram_parameter("Wp", [F, OUTF], fp16, isOutput=False)
    b_in = [
        nc.declare_dram_parameter(f"b{i}", [F, 1], fp32, isOutput=False)
        for i in (1, 2, 3)
    ]
    bp_in = nc.declare_dram_parameter("bp", [OUTF, 1], fp32, isOutput=False)
    g_in = [
        nc.declare_dram_parameter(f"g{i}", [F, 1], fp32, isOutput=False) for i in (1, 2)
    ]
    be_in = [
        nc.declare_dram_parameter(f"be{i}", [F, 1], fp32, isOutput=False)
        for i in (1, 2)
    ]
    out_ext = nc.declare_dram_parameter("outT", [OUTF, SHARD], fp32, isOutput=True)

    with tile.TileContext(nc) as tc:
        from contextlib import ExitStack

        with ExitStack() as ctx:
            dram = ctx.enter_context(tc.tile_pool(name="dram", bufs=1, space="DRAM"))
            singles = ctx.enter_context(tc.tile_pool(name="singles", bufs=1))
            glo_p = ctx.enter_context(tc.tile_pool(name="glo", bufs=6))
            ghi_p = ctx.enter_context(tc.tile_pool(name="ghi", bufs=6))
            s_p = ctx.enter_context(tc.tile_pool(name="spool", bufs=4))
            conv_ps = ctx.enter_context(tc.tile_pool(name="convps", bufs=3, space="PSUM"))
            z_ps = ctx.enter_context(tc.tile_pool(name="zps", bufs=2, space="PSUM"))
            t_ps = ctx.enter_context(tc.tile_pool(name="tps", bufs=2, space="PSUM"))
            zstage = ctx.enter_context(tc.tile_pool(name="zstage", bufs=2))
            rstage = ctx.enter_context(tc.tile_pool(name="rstage", bufs=3))
            small = ctx.enter_context(tc.tile_pool(name="small", bufs=2))

            # DRAM internals
            z_shardAs = [dram.tile([ASPLIT, F], fp16, name=f"z_shardA{i}") for i in range(3)]
            z_shardBs = [dram.tile([BSPLIT, F], fp16, name=f"z_shardB{i}") for i in range(3)]
            # +8 rows: barrier-AllGather output lands there so gathers (whose
            # source AP is the full tensor) depend on every core's publish
            z_fullAs = [
                dram.tile([N_CORES * ASPLIT + 8, F], fp16, addr_space="Shared", name=f"z_fullA{i}")
                for i in range(3)
            ]
            z_fullBs = [
                dram.tile([N_CORES * BSPLIT + 8, F], fp16, addr_space="Shared", name=f"z_fullB{i}")
                for i in range(3)
            ]

            stats_locs = [dram.tile([F, 2], fp32, name=f"stats_loc{i}") for i in range(2)]
            stats_globs = [dram.tile([F, 2], fp32, addr_space="Shared", name=f"stats_glob{i}") for i in range(2)]

            # ---- load constants (x and W1 first: z1 starts immediately) ----
            actA = singles.tile([F, SHARD], fp16)  # layer input act^T
            nc.sync.dma_start(out=actA[:], in_=xT_in[:])
            w_sb = []
            for i in range(3):
                w = singles.tile([F, F], fp16, name=f"w{i}")
                nc.sync.dma_start(out=w[:], in_=w_in[i][:])
                w_sb.append(w)
            idx_sb = singles.tile([128, total_slots // 16], i16)
            nc.sync.dma_start(out=idx_sb[:], in_=idx_in[:])
            wp_sb = singles.tile([F, OUTF], fp16)
            nc.sync.dma_start(out=wp_sb[:], in_=wp_in[:])
            b_sb = []
            for i in range(3):
                b = singles.tile([F, 1], fp32, name=f"b{i}")
                nc.sync.dma_start(out=b[:], in_=b_in[i][:])
                b_sb.append(b)
            bp_sb = singles.tile([OUTF, 1], fp32)
            nc.sync.dma_start(out=bp_sb[:], in_=bp_in[:])
            g_sb, be_sb = [], []
            for i in range(2):
                g = singles.tile([F, 1], fp32, name=f"g{i}")
                nc.sync.dma_start(out=g[:], in_=g_in[i][:])
                g_sb.append(g)
                be = singles.tile([F, 1], fp32, name=f"be{i}")
                nc.sync.dma_start(out=be[:], in_=be_in[i][:])
                be_sb.append(be)

            # identity matrix for PE transpose (fp16 to pair with fp16 lhsT)
            ident_sb = singles.tile([128, 128], fp16)
            identi32 = singles.tile([128, 128], mybir.dt.int32)
            nc.gpsimd.iota(
                identi32[:], pattern=[[1, 128]], base=0, channel_multiplier=-1
            )
            nc.vector.tensor_scalar(
                out=ident_sb[:],
                in0=identi32[:],
                scalar1=0.0,
                scalar2=None,
                op0=OP.is_equal,
            )

            # persistent activations
            actB = singles.tile([F, SHARD], fp16)
            conv_sb = singles.tile([F, SHARD], fp32)
            sumcols = singles.tile([F, NTILE], fp32)
            sqcols = singles.tile([F, NTILE], fp32)
            sq_scratch = singles.tile([F, TILE], fp32)
            # row-major local z (lhsT for the self-loop diagonal blocks)
            z_rows = singles.tile([128, NTILE * F], fp16)
            nc.vector.memset(z_rows[:], 0.0)

            # persist the leading tiles' S chunks in SBUF across layers
            nsb_per_tile = [
                1 + int(pad_blocks[t, 0]) + int(pad_blocks[t, 1])
                for t in range(NTILE)
            ]
            S_PERS_BYTES = int(os.environ.get("KGNN_SPERS", str(8 * 1024 * 1024)))
            pers_t = 0
            acc = 0
            for t in range(NTILE):
                sz = nsb_per_tile[t] * TILE * 128 * 2
                if acc + sz > S_PERS_BYTES:
                    break
                acc += sz
                pers_t += 1
            s_pers = [
                singles.tile([128, nsb_per_tile[t] * TILE], fp16, name=f"spers{t}")
                for t in range(pers_t)
            ]

            zchunks = []
            o = 0
            while o < SHARD:
                w = min(ZCHUNK, SHARD - o)
                zchunks.append((o, w))
                o += w

            def publish(z_shard, z_full, rows):
                """AllGather the shard half into the Shared z_full."""
                nc.gpsimd.collective_compute(
                    "AllGather",
                    mybir.AluOpType.bypass,
                    replica_groups=[list(range(N_CORES))],
                    ins=[z_shard[:, :].opt()],
                    outs=[z_full[0 : N_CORES * rows, :].opt()],
                )

            def produce_z(act_src, w_idx):
                """z^T = W @ act^T, transpose into z_rows, DMA shard halves,
                publish A while B rows are still being produced."""

                def emit_chunks(chunks):
                    for (o, w) in chunks:
                        zp = z_ps.tile([F, ZCHUNK], fp32, tag="zps")
                        nc.tensor.matmul(
                            zp[:, :w], lhsT=w_sb[w_idx][:], rhs=act_src[:, o : o + w],
                            start=True, stop=True,
                        )
                        zs = zstage.tile([F, ZCHUNK], fp16, tag="zstage")
                        nc.scalar.copy(zs[:, :w], zp[:, :w])
                        k = 0
                        while k < w:
                            wk = min(128, w - k)
                            tp = t_ps.tile([128, 128], fp16, tag="tps")
                            nc.tensor.transpose(
                                tp[:wk, :], zs[:, k : k + wk], ident_sb[:]
                            )
                            # z_rows free dim is tile-major: tile t at cols t*F
                            t = (o + k) // TILE
                            nc.vector.tensor_copy(
                                z_rows[:wk, t * F : (t + 1) * F], tp[:wk, :]
                            )
                            if o + k < ASPLIT:
                                dst = z_shardAs[w_idx][o + k : o + k + wk, :]
                            else:
                                dst = z_shardBs[w_idx][
                                    o + k - ASPLIT : o + k - ASPLIT + wk, :
                                ]
                            nc.sync.dma_start(
                                out=dst,
                                in_=z_rows[:wk, t * F : (t + 1) * F],
                            )
                            k += wk

                nA = ASPLIT // ZCHUNK  # 5 chunks cover rows [0, ASPLIT)
                emit_chunks(zchunks[:nA])
                publish(z_shardAs[w_idx], z_fullAs[w_idx], ASPLIT)
                emit_chunks(zchunks[nA:])
                publish(z_shardBs[w_idx], z_fullBs[w_idx], BSPLIT)

            qrr = [0]  # round-robin SWDGE queue counter

            def conv_layer(lyr):
                """gather + S-matmul segment sum into conv_sb; bias; stats cols.

                Gathers use the prepare/trigger protocol: descriptor
                generation (prep) has no data dependency on z_full, so it
                runs on the Pool engine during the AllGather; the trigger
                carries the deferred z_full read dependency."""
                for gi, ginfo in enumerate(groups):
                    lo = ginfo["lo"]
                    hi = ginfo["hi"]
                    blockmap = {}
                    for nm, run in (("lo", lo), ("hi", hi)):
                        pool = glo_p if nm == "lo" else ghi_p
                        src_ap = (
                            z_fullAs[lyr][:, :] if nm == "lo" else z_fullBs[lyr][:, :]
                        )
                        for (cs0, cns) in run["calls"]:
                            nblk = cns // TILE
                            gbuf = pool.tile([128, nblk, F], fp16, tag=f"g{nm}")
                            nc.gpsimd.dma_gather(
                                gbuf[:],
                                src_ap,
                                idx_sb[:, cs0 // 16 : (cs0 + cns) // 16],
                                cns,
                                cns,
                                F,
                                queue_num=qrr[0] % NQ,
                            )
                            qrr[0] += 1
                            for j in range(nblk):
                                blockmap[cs0 // TILE + j] = (gbuf, j)
                    for ti, t in enumerate(ginfo["tiles"]):
                        # blocks for tile t in S-stream order (lo then hi)
                        blocks = []
                        for nm, run in (("lo", lo), ("hi", hi)):
                            s0, nb = run["tile_blocks"][ti]
                            for j in range(nb):
                                gb = s0 // TILE + j
                                blocks.append(blockmap[gb])
                        sb_t = int(sblock_start[t])
                        nsb_t = 1 + len(blocks)
                        if t < pers_t:
                            s_sb = s_pers[t]
                            if lyr == 0:
                                nc.sync.dma_start(
                                    out=s_sb[:],
                                    in_=smat_in[:, sb_t * TILE : (sb_t + nsb_t) * TILE],
                                )
                        else:
                            s_sb = s_p.tile([128, max_sb * TILE], fp16, tag="smat")
                            nc.sync.dma_start(
                                out=s_sb[:, : nsb_t * TILE],
                                in_=smat_in[:, sb_t * TILE : (sb_t + nsb_t) * TILE],
                            )
                        cps = conv_ps.tile([F, TILE], fp32, tag="convps")
                        # self-loop diagonal block first
                        nc.tensor.matmul(
                            cps[:],
                            lhsT=z_rows[:, t * F : (t + 1) * F],
                            rhs=s_sb[:, 0:TILE],
                            start=True,
                            stop=(len(blocks) == 0),
                        )
                        for bi, (gbuf, lb) in enumerate(blocks):
                            nc.tensor.matmul(
                                cps[:],
                                lhsT=gbuf[:, lb, :],
                                rhs=s_sb[:, (1 + bi) * TILE : (2 + bi) * TILE],
                                start=False,
                                stop=(bi == len(blocks) - 1),
                            )
                        tw = LAST_W if t == NTILE - 1 else TILE
                        o = t * TILE
                        nc.scalar.activation(
                            out=conv_sb[:, o : o + tw],
                            in_=cps[:, :tw],
                            func=AF.Identity,
                            bias=b_sb[lyr][:],
                            scale=1.0,
                            accum_out=sumcols[:, t : t + 1],
                        )
                        nc.scalar.activation(
                            out=sq_scratch[:, :tw],
                            in_=conv_sb[:, o : o + tw],
                            func=AF.Square,
                            accum_out=sqcols[:, t : t + 1],
                        )

            def bn_relu(lyr, act_out):
                """global BN stats allreduce + fused scale/shift/relu -> act_out fp16."""
                ssum = small.tile([F, 1], fp32, tag="ssum")
                nc.vector.tensor_reduce(
                    ssum[:], sumcols[:], axis=mybir.AxisListType.X, op=OP.add
                )
                ssq = small.tile([F, 1], fp32, tag="ssq")
                nc.vector.tensor_reduce(
                    ssq[:], sqcols[:], axis=mybir.AxisListType.X, op=OP.add
                )
                st = small.tile([F, 2], fp32, tag="stats")
                nc.vector.tensor_copy(st[:, 0:1], ssum[:])
                nc.vector.tensor_copy(st[:, 1:2], ssq[:])
                nc.sync.dma_start(out=stats_locs[lyr][:], in_=st[:])
                nc.gpsimd.collective_compute(
                    "AllReduce",
                    OP.add,
                    replica_groups=[list(range(N_CORES))],
                    ins=[stats_locs[lyr][:].opt()],
                    outs=[stats_globs[lyr][:].opt()],
                )
                stg = small.tile([F, 2], fp32, tag="statsg")
                nc.sync.dma_start(out=stg[:], in_=stats_globs[lyr][:])
                mean = small.tile([F, 1], fp32, tag="mean")
                nc.vector.tensor_scalar_mul(mean[:], stg[:, 0:1], 1.0 / N)
                ex2 = small.tile([F, 1], fp32, tag="ex2")
                nc.vector.tensor_scalar_mul(ex2[:], stg[:, 1:2], 1.0 / N)
                var = small.tile([F, 1], fp32, tag="var")
                nc.vector.tensor_tensor(var[:], mean[:], mean[:], op=OP.mult)
                nc.vector.tensor_sub(var[:], ex2[:], var[:])
                nc.vector.tensor_scalar_add(var[:], var[:], BN_EPS)
                std = small.tile([F, 1], fp32, tag="std")
                nc.scalar.sqrt(std[:], var[:])
                rstd = small.tile([F, 1], fp32, tag="rstd")
                nc.vector.reciprocal(rstd[:], std[:])
                scale = small.tile([F, 1], fp32, tag="scale")
                nc.vector.tensor_mul(scale[:], rstd[:], g_sb[lyr][:])
                shift = small.tile([F, 1], fp32, tag="shift")
                nc.vector.tensor_mul(shift[:], mean[:], scale[:])
                nc.vector.tensor_sub(shift[:], be_sb[lyr][:], shift[:])
                # chunked so downstream z-production starts on early chunks
                for (o, w) in zchunks:
                    nc.scalar.activation(
                        out=act_out[:, o : o + w],
                        in_=conv_sb[:, o : o + w],
                        func=AF.Relu,
                        bias=shift[:],
                        scale=scale[:],
                    )

            PHASE = int(os.environ.get("KGNN_PHASE", "9"))
            nc.vector.memset(actB[:], 0.0)
            nc.vector.memset(conv_sb[:], 0.0)
            # ---- layer 1 ----
            if PHASE >= 1:
                with nc.named_scope("z1"):
                    produce_z(actA, 0)
            if PHASE >= 2:
                with nc.named_scope("conv1"):
                    conv_layer(0)
            if PHASE >= 3:
                with nc.named_scope("bn1"):
                    bn_relu(0, actB)
            if PHASE >= 4:
                # ---- layer 2 ----
                with nc.named_scope("z2"):
                    produce_z(actB, 1)
                with nc.named_scope("conv2"):
                    conv_layer(1)
                with nc.named_scope("bn2"):
                    bn_relu(1, actA)
                    # jk12 = max(act1, act2) -> actB
                    nc.vector.tensor_max(actB[:], actB[:], actA[:])
            if PHASE >= 5:
                # ---- layer 3 ----
                with nc.named_scope("z3"):
                    produce_z(actA, 2)
                with nc.named_scope("conv3"):
                    conv_layer(2)
            # ---- JK max + projection, chunked to overlap conv3's tail ----
            for (o, w) in zchunks:
                # conv3 -> fp16, jk = max(jk12, conv3), project
                nc.scalar.copy(actA[:, o : o + w], conv_sb[:, o : o + w])
                nc.vector.tensor_max(
                    actB[:, o : o + w], actB[:, o : o + w], actA[:, o : o + w]
                )
                pp = z_ps.tile([F, ZCHUNK], fp32, tag="zps")
                nc.tensor.matmul(
                    pp[:OUTF, :w], lhsT=wp_sb[:], rhs=actB[:, o : o + w],
                    start=True, stop=True,
                )
                po = rstage.tile([OUTF, ZCHUNK], fp32, tag="pout")
                nc.scalar.activation(
                    out=po[:, :w], in_=pp[:OUTF, :w], func=AF.Identity,
                    bias=bp_sb[:], scale=1.0,
                )
                nc.sync.dma_start(out=out_ext[:, o : o + w], in_=po[:, :w])

    nc.compile()
    return nc


_CACHE = {}
_last_nc = None
_last_in_maps = None


def kernel(**inputs):
    global _last_nc, _last_in_maps
    from concourse.bass_utils import run_bass_kernel_spmd

    x = np.asarray(inputs["x"], dtype=np.float32)
    edge_index = np.asarray(inputs["edge_index"])

    ck = hash(edge_index.tobytes())
    if ck not in _CACHE:
        structure, per_core = _preprocess(edge_index)
        nc = _build(structure)
        _CACHE[ck] = (structure, per_core, nc)
    structure, per_core, nc = _CACHE[ck]

    in_maps = []
    for c in range(N_CORES):
        xc = x[c * SHARD : (c + 1) * SHARD].astype(np.float16)
        m = {
            "xT": np.ascontiguousarray(xc.T),
            "idx": per_core[c]["idx"],
            "smat": per_core[c]["smat"],
            "W1": np.asarray(inputs["W1"], np.float16),
            "W2": np.asarray(inputs["W2"], np.float16),
            "W3": np.asarray(inputs["W3"], np.float16),
            "Wp": np.asarray(inputs["Wp"], np.float16),
            "b1": np.asarray(inputs["b1"], np.float32).reshape(F, 1),
            "b2": np.asarray(inputs["b2"], np.float32).reshape(F, 1),
            "b3": np.asarray(inputs["b3"], np.float32).reshape(F, 1),
            "bp": np.asarray(inputs["bp"], np.float32).reshape(OUTF, 1),
            "g1": np.asarray(inputs["g1"], np.float32).reshape(F, 1),
            "g2": np.asarray(inputs["g2"], np.float32).reshape(F, 1),
            "be1": np.asarray(inputs["be1"], np.float32).reshape(F, 1),
            "be2": np.asarray(inputs["be2"], np.float32).reshape(F, 1),
        }
        in_maps.append(m)

    _last_nc, _last_in_maps = nc, in_maps
    res = run_bass_kernel_spmd(nc, in_maps, core_ids=list(range(N_CORES)))
    out = np.empty((N, OUTF), dtype=np.float32)
    for c in range(N_CORES):
        out[c * SHARD : (c + 1) * SHARD] = res.results[c]["outT"].T
    return out


# revision 38
# speedup vs baseline: 1.0759x; 1.0759x over previous
"""GCN+JumpingKnowledge distributed Trainium2 kernel (8 NeuronCores).

Strategy: shard destination nodes across 8 cores (6250 each). Per layer:
  - sharded feature transform z^T = W @ act^T on TensorE; transpose to
    row-major into persistent SBUF z_rows, DMA to HBM shard halves,
    A/B-split AllGather into Shared z_fullA/z_fullB fp16 (AG-A fires
    while B rows are still being produced; gathers sourced from A
    overlap AG-B's transfer; rebased indices keep both halves < 2^15
    for int16 gather indices)
  - dma_gather source rows for this core's edges (sorted by dst tile,
    split by A/B half, padded to a common per-(tile,half) block count
    across cores so one SPMD program fits all); gather calls
    round-robin over 4 SWDGE queues (desc-gen overlaps ~2x)
  - segment-sum via TensorE: psum[feat,dst] += G_blk^T @ S_blk where
    S_blk[e,d] = (dst_e == d) * normval_e is precomputed on host and
    streamed from HBM (layer-invariant; the leading ~8MB is persisted
    in SBUF across layers); self-loops are a diagonal S block whose
    lhsT is the local z_rows (no gather)
  - BN stats via per-tile accumulators + 1KB AllReduce, chunked
    scale/shift/ReLU on ACT so z-production starts early; JK max and
    final projection chunked to overlap conv3's tail.
"""

import os
import sys

import numpy as np

sys.path.insert(0, "/opt/trn_rl_repo")

N = 50000
E = 800000
F = 128
OUTF = 64
N_CORES = 8
SHARD = N // N_CORES  # 6250
TILE = 128
NTILE = (SHARD + TILE - 1) // TILE  # 49
LAST_W = SHARD - (NTILE - 1) * TILE  # 106
ASPLIT = 2560  # rows-per-shard split for the A/B AllGather (5 z-chunks)
BSPLIT = SHARD - ASPLIT  # 3690
GRP = 4  # tiles per gather group
BN_EPS = 1e-5
ZCHUNK = 512
NQ = 4  # SWDGE queues
MAX_CALL = 1024  # hw limit: 1024 idxs per dma_gather call


def _preprocess(edge_index):
    """Host-side edge routing. Returns (structure, per_core_arrays)."""
    src = np.asarray(edge_index[0], dtype=np.int64)
    dst = np.asarray(edge_index[1], dtype=np.int64)

    deg = np.bincount(dst, minlength=N).astype(np.float64) + 1.0
    dinv = 1.0 / np.sqrt(deg)
    dinv2 = (dinv * dinv).astype(np.float32)  # self-loop weights

    normval = (dinv[src] * dinv[dst]).astype(np.float32)

    core = dst // SHARD
    tile_id = (dst % SHARD) // TILE
    src_core = src // SHARD
    src_row = src % SHARD
    half = (src_row >= ASPLIT).astype(np.int64)  # 0 -> z_fullA, 1 -> z_fullB
    gidx = np.where(
        half == 0, src_core * ASPLIT + src_row, src_core * BSPLIT + src_row - ASPLIT
    )
    dstoff = ((dst % SHARD) % TILE).astype(np.int64)

    # per (core, tile, half) counts, padded to the max across cores
    key = (core * NTILE + tile_id) * 2 + half
    counts = np.bincount(key, minlength=N_CORES * NTILE * 2).reshape(
        N_CORES, NTILE, 2
    )
    maxcnt = counts.max(axis=0)  # [NTILE, 2]
    pad_blocks = (maxcnt + TILE - 1) // TILE  # blocks per (tile, half)

    # slot layout: groups of GRP tiles; per group all lo slots then all hi.
    groups = []
    slot_start = np.zeros((NTILE, 2), dtype=np.int64)
    cursor = 0
    for g0 in range(0, NTILE, GRP):
        tiles = list(range(g0, min(g0 + GRP, NTILE)))
        ginfo = {"tiles": tiles}
        for h, nm in ((0, "lo"), (1, "hi")):
            run_slot0 = cursor
            tb = []
            for t in tiles:
                slot_start[t, h] = cursor
                tb.append((cursor, int(pad_blocks[t, h])))
                cursor += int(pad_blocks[t, h]) * TILE
            run_slots = cursor - run_slot0
            calls = []
            o = run_slot0
            while o < run_slot0 + run_slots:
                n = min(MAX_CALL, run_slot0 + run_slots - o)
                calls.append((o, n))
                o += n
            ginfo[nm] = {
                "slot0": run_slot0,
                "nslots": run_slots,
                "tile_blocks": tb,
                "calls": calls,
            }
        groups.append(ginfo)
    total_slots = cursor
    total_blocks = total_slots // TILE

    # S stream layout: tile-major [diag, lo blocks, hi blocks] per tile.
    sblock_start = np.zeros(NTILE, dtype=np.int64)
    nsb = 0
    for t in range(NTILE):
        sblock_start[t] = nsb
        nsb += 1 + int(pad_blocks[t, 0]) + int(pad_blocks[t, 1])
    # map global gather block -> S stream block
    gblk2sblk = np.zeros(total_blocks, dtype=np.int64)
    for t in range(NTILE):
        for h in (0, 1):
            s0 = slot_start[t, h] // TILE
            base = sblock_start[t] + 1 + (int(pad_blocks[t, 0]) if h else 0)
            for j in range(int(pad_blocks[t, h])):
                gblk2sblk[s0 + j] = base + j

    # per-core slot content + S matrices
    per_core = []
    for c in range(N_CORES):
        m = core == c
        e_t = tile_id[m]
        e_h = half[m]
        e_src = gidx[m]
        e_nv = normval[m]
        e_do = dstoff[m]
        order = np.lexsort((e_h, e_t))
        e_t, e_h = e_t[order], e_h[order]
        e_src, e_nv, e_do = e_src[order], e_nv[order], e_do[order]
        # rank within (t, h) group
        k = e_t * 2 + e_h
        cnt_c = np.bincount(k, minlength=NTILE * 2)
        grp_starts = np.concatenate([[0], np.cumsum(cnt_c)[:-1]])
        rank = np.arange(len(k)) - grp_starts[k]
        slots = slot_start[e_t, e_h] + rank

        idx_vals = np.zeros(total_slots, dtype=np.int16)
        idx_vals[slots] = e_src.astype(np.int16)

        # idx wrapped layout: slot i -> partition i%16 (replicated x8), col i//16
        idx_arr = np.zeros((128, total_slots // 16), dtype=np.int16)
        v16 = idx_vals.reshape(-1, 16).T  # [16, total/16]
        for g in range(8):
            idx_arr[16 * g : 16 * g + 16] = v16

        # S matrices: [nsb, 128 e, 128 d] fp16
        smat = np.zeros((nsb, TILE, TILE), dtype=np.float16)
        e_sblk = gblk2sblk[slots // TILE]
        smat[e_sblk, slots % TILE, e_do] = e_nv.astype(np.float16)
        # diagonal self-loop blocks
        node0 = c * SHARD
        dia = np.arange(TILE)
        for t in range(NTILE):
            tw = LAST_W if t == NTILE - 1 else TILE
            blk = smat[sblock_start[t]]
            blk[dia[:tw], dia[:tw]] = dinv2[node0 + t * TILE : node0 + t * TILE + tw]
        # -> [128 e, nsb * 128 d] for DMA-friendly streaming
        smat_flat = np.ascontiguousarray(
            np.transpose(smat, (1, 0, 2)).reshape(TILE, nsb * TILE)
        )
        per_core.append({"idx": idx_arr, "smat": smat_flat})

    structure = {
        "groups": groups,
        "total_slots": total_slots,
        "total_blocks": total_blocks,
        "nsb": nsb,
        "sblock_start": sblock_start,
        "pad_blocks": pad_blocks,
    }
    return structure, per_core


def _build(structure):
    import concourse.bacc as bacc
    import concourse.tile as tile
    from concourse import mybir
    import concourse.bass as bass

    fp32 = mybir.dt.float32
    fp16 = mybir.dt.float16
    i16 = mybir.dt.int16
    AF = mybir.ActivationFunctionType
    OP = mybir.AluOpType

    groups = structure["groups"]
    total_slots = structure["total_slots"]
    nsb = structure["nsb"]
    sblock_start = structure["sblock_start"]
    pad_blocks = structure["pad_blocks"]
    max_sb = 1 + int(pad_blocks.sum(axis=1).max())  # largest per-tile S chunk

    nc = bacc.Bacc(
        "TRN2", target_bir_lowering=False, num_devices=N_CORES, num_swdge_queues=NQ
    )

    # ---- I/O ----
    xT_in = nc.declare_dram_parameter("xT", [F, SHARD], fp16, isOutput=False)
    idx_in = nc.declare_dram_parameter("idx", [128, total_slots // 16], i16, isOutput=False)
    smat_in = nc.declare_dram_parameter("smat", [128, nsb * TILE], fp16, isOutput=False)
    w_in = [
        nc.declare_dram_parameter(f"W{i}", [F, F], fp16, isOutput=False)
        for i in (1, 2, 3)
    ]
    wp_in = nc.declare_dram_parameter("Wp", [F, OUTF], fp16, isOutput=False)
    b_in = [
        nc.declare_dram_parameter(f"b{i}", [F, 1], fp32, isOutput=False)
        for i in (1, 2, 3)
    ]
    bp_in = nc.declare_dram_parameter("bp", [OUTF, 1], fp32, isOutput=False)
    g_in = [
        nc.declare_dram_parameter(f"g{i}", [F, 1], fp32, isOutput=False) for i in (1, 2)
    ]
    be_in = [
        nc.declare_dram_parameter(f"be{i}", [F, 1], fp32, isOutput=False)
        for i in (1, 2)
    ]
    out_ext = nc.declare_dram_parameter("outT", [OUTF, SHARD], fp32, isOutput=True)

    with tile.TileContext(nc) as tc:
        from contextlib import ExitStack

        with ExitStack() as ctx:
            dram = ctx.enter_context(tc.tile_pool(name="dram", bufs=1, space="DRAM"))
            singles = ctx.enter_context(tc.tile_pool(name="singles", bufs=1))
            glo_p = ctx.enter_context(tc.tile_pool(name="glo", bufs=6))
            ghi_p = ctx.enter_context(tc.tile_pool(name="ghi", bufs=6))
            s_p = ctx.enter_context(tc.tile_pool(name="spool", bufs=4))
            conv_ps = ctx.enter_context(tc.tile_pool(name="convps", bufs=3, space="PSUM"))
            z_ps = ctx.enter_context(tc.tile_pool(name="zps", bufs=2, space="PSUM"))
            t_ps = ctx.enter_context(tc.tile_pool(name="tps", bufs=2, space="PSUM"))
            zstage = ctx.enter_context(tc.tile_pool(name="zstage", bufs=2))
            rstage = ctx.enter_context(tc.tile_pool(name="rstage", bufs=3))
            small = ctx.enter_context(tc.tile_pool(name="small", bufs=2))

            # DRAM internals
            z_shardAs = [dram.tile([ASPLIT, F], fp16, name=f"z_shardA{i}") for i in range(3)]
            z_shardBs = [dram.tile([BSPLIT, F], fp16, name=f"z_shardB{i}") for i in range(3)]
            # +8 rows: barrier-AllGather output lands there so gathers (whose
            # source AP is the full tensor) depend on every core's publish
            z_fullAs = [
                dram.tile([N_CORES * ASPLIT + 8, F], fp16, addr_space="Shared", name=f"z_fullA{i}")
                for i in range(3)
            ]
            z_fullBs = [
                dram.tile([N_CORES * BSPLIT + 8, F], fp16, addr_space="Shared", name=f"z_fullB{i}")
                for i in range(3)
            ]

            stats_locs = [dram.tile([F, 2], fp32, name=f"stats_loc{i}") for i in range(2)]
            stats_globs = [dram.tile([F, 2], fp32, addr_space="Shared", name=f"stats_glob{i}") for i in range(2)]

            # ---- load constants (x and W1 first: z1 starts immediately) ----
            actA = singles.tile([F, SHARD], fp16)  # layer input act^T
            nc.sync.dma_start(out=actA[:], in_=xT_in[:])
            w_sb = []
            for i in range(3):
                w = singles.tile([F, F], fp16, name=f"w{i}")
                nc.sync.dma_start(out=w[:], in_=w_in[i][:])
                w_sb.append(w)
            idx_sb = singles.tile([128, total_slots // 16], i16)
            nc.sync.dma_start(out=idx_sb[:], in_=idx_in[:])
            wp_sb = singles.tile([F, OUTF], fp16)
            nc.sync.dma_start(out=wp_sb[:], in_=wp_in[:])
            b_sb = []
            for i in range(3):
                b = singles.tile([F, 1], fp32, name=f"b{i}")
                nc.sync.dma_start(out=b[:], in_=b_in[i][:])
                b_sb.append(b)
            bp_sb = singles.tile([OUTF, 1], fp32)
            nc.sync.dma_start(out=bp_sb[:], in_=bp_in[:])
            g_sb, be_sb = [], []
            for i in range(2):
                g = singles.tile([F, 1], fp32, name=f"g{i}")
                nc.sync.dma_start(out=g[:], in_=g_in[i][:])
                g_sb.append(g)
                be = singles.tile([F, 1], fp32, name=f"be{i}")
                nc.sync.dma_start(out=be[:], in_=be_in[i][:])
                be_sb.append(be)

            # identity matrix for PE transpose (fp16 to pair with fp16 lhsT)
            ident_sb = singles.tile([128, 128], fp16)
            identi32 = singles.tile([128, 128], mybir.dt.int32)
            nc.gpsimd.iota(
                identi32[:], pattern=[[1, 128]], base=0, channel_multiplier=-1
            )
            nc.vector.tensor_scalar(
                out=ident_sb[:],
                in0=identi32[:],
                scalar1=0.0,
                scalar2=None,
                op0=OP.is_equal,
            )

            # persistent activations
            actB = singles.tile([F, SHARD], fp16)
            conv_sb = singles.tile([F, SHARD], fp32)
            sumcols = singles.tile([F, NTILE], fp32)
            sqcols = singles.tile([F, NTILE], fp32)
            sq_scratch = singles.tile([F, TILE], fp32)
            # row-major local z (lhsT for the self-loop diagonal blocks)
            z_rows = singles.tile([128, NTILE * F], fp16)
            nc.vector.memset(z_rows[:], 0.0)

            # persist the leading tiles' S chunks in SBUF across layers
            nsb_per_tile = [
                1 + int(pad_blocks[t, 0]) + int(pad_blocks[t, 1])
                for t in range(NTILE)
            ]
            S_PERS_BYTES = int(os.environ.get("KGNN_SPERS", str(8 * 1024 * 1024)))
            pers_t = 0
            acc = 0
            for t in range(NTILE):
                sz = nsb_per_tile[t] * TILE * 128 * 2
                if acc + sz > S_PERS_BYTES:
                    break
                acc += sz
                pers_t += 1
            s_pers = [
                singles.tile([128, nsb_per_tile[t] * TILE], fp16, name=f"spers{t}")
                for t in range(pers_t)
            ]

            zchunks = []
            o = 0
            while o < SHARD:
                w = min(ZCHUNK, SHARD - o)
                zchunks.append((o, w))
                o += w

            def publish(z_shard, z_full, rows):
                """AllGather the shard half into the Shared z_full."""
                nc.gpsimd.collective_compute(
                    "AllGather",
                    mybir.AluOpType.bypass,
                    replica_groups=[list(range(N_CORES))],
                    ins=[z_shard[:, :].opt()],
                    outs=[z_full[0 : N_CORES * rows, :].opt()],
                )

            def produce_z(act_src, w_idx):
                """z^T = W @ act^T, transpose into z_rows, DMA shard halves,
                publish A while B rows are still being produced."""

                def emit_chunks(chunks):
                    for (o, w) in chunks:
                        zp = z_ps.tile([F, ZCHUNK], fp32, tag="zps")
                        nc.tensor.matmul(
                            zp[:, :w], lhsT=w_sb[w_idx][:], rhs=act_src[:, o : o + w],
                            start=True, stop=True,
                        )
                        zs = zstage.tile([F, ZCHUNK], fp16, tag="zstage")
                        nc.scalar.copy(zs[:, :w], zp[:, :w])
                        k = 0
                        while k < w:
                            wk = min(128, w - k)
                            tp = t_ps.tile([128, 128], fp16, tag="tps")
                            nc.tensor.transpose(
                                tp[:wk, :], zs[:, k : k + wk], ident_sb[:]
                            )
                            # z_rows free dim is tile-major: tile t at cols t*F
                            t = (o + k) // TILE
                            nc.vector.tensor_copy(
                                z_rows[:wk, t * F : (t + 1) * F], tp[:wk, :]
                            )
                            if o + k < ASPLIT:
                                dst = z_shardAs[w_idx][o + k : o + k + wk, :]
                            else:
                                dst = z_shardBs[w_idx][
                                    o + k - ASPLIT : o + k - ASPLIT + wk, :
                                ]
                            nc.sync.dma_start(
                                out=dst,
                                in_=z_rows[:wk, t * F : (t + 1) * F],
                            )
                            k += wk

                nA = ASPLIT // ZCHUNK  # 5 chunks cover rows [0, ASPLIT)
                emit_chunks(zchunks[:nA])
                publish(z_shardAs[w_idx], z_fullAs[w_idx], ASPLIT)
                emit_chunks(zchunks[nA:])
                publish(z_shardBs[w_idx], z_fullBs[w_idx], BSPLIT)

            qrr = [0]  # round-robin SWDGE queue counter

            def conv_layer(lyr):
                """gather + S-matmul segment sum into conv_sb; bias; stats cols.

                Gathers use the prepare/trigger protocol: descriptor
                generation (prep) has no data dependency on z_full, so it
                runs on the Pool engine during the AllGather; the trigger
                carries the deferred z_full read dependency."""
                for gi, ginfo in enumerate(groups):
                    lo = ginfo["lo"]
                    hi = ginfo["hi"]
                    blockmap = {}
                    for nm, run in (("lo", lo), ("hi", hi)):
                        pool = glo_p if nm == "lo" else ghi_p
                        src_ap = (
                            z_fullAs[lyr][:, :] if nm == "lo" else z_fullBs[lyr][:, :]
                        )
                        for (cs0, cns) in run["calls"]:
                            nblk = cns // TILE
                            gbuf = pool.tile([128, nblk, F], fp16, tag=f"g{nm}")
                            nc.gpsimd.dma_gather(
                                gbuf[:],
                                src_ap,
                                idx_sb[:, cs0 // 16 : (cs0 + cns) // 16],
                                cns,
                                cns,
                                F,
                                queue_num=qrr[0] % NQ,
                            )
                            qrr[0] += 1
                            for j in range(nblk):
                                blockmap[cs0 // TILE + j] = (gbuf, j)
                    for ti, t in enumerate(ginfo["tiles"]):
                        # blocks for tile t in S-stream order (lo then hi)
                        blocks = []
                        for nm, run in (("lo", lo), ("hi", hi)):
                            s0, nb = run["tile_blocks"][ti]
                            for j in range(nb):
                                gb = s0 // TILE + j
                                blocks.append(blockmap[gb])
                        sb_t = int(sblock_start[t])
                        nsb_t = 1 + len(blocks)
                        if t < pers_t:
                            s_sb = s_pers[t]
                            if lyr == 0:
                                nc.sync.dma_start(
                                    out=s_sb[:],
                                    in_=smat_in[:, sb_t * TILE : (sb_t + nsb_t) * TILE],
                                )
                        else:
                            s_sb = s_p.tile([128, max_sb * TILE], fp16, tag="smat")
                            nc.sync.dma_start(
                                out=s_sb[:, : nsb_t * TILE],
                                in_=smat_in[:, sb_t * TILE : (sb_t + nsb_t) * TILE],
                            )
                        cps = conv_ps.tile([F, TILE], fp32, tag="convps")
                        # self-loop diagonal block first
                        nc.tensor.matmul(
                            cps[:],
                            lhsT=z_rows[:, t * F : (t + 1) * F],
                            rhs=s_sb[:, 0:TILE],
                            start=True,
                            stop=(len(blocks) == 0),
                        )
                        for bi, (gbuf, lb) in enumerate(blocks):
                            nc.tensor.matmul(
                                cps[:],
                                lhsT=gbuf[:, lb, :],
                                rhs=s_sb[:, (1 + bi) * TILE : (2 + bi) * TILE],
                                start=False,
                                stop=(bi == len(blocks) - 1),
                            )
                        tw = LAST_W if t == NTILE - 1 else TILE
                        o = t * TILE
                        nc.scalar.activation(
                            out=conv_sb[:, o : o + tw],
                            in_=cps[:, :tw],
                            func=AF.Identity,
                            bias=b_sb[lyr][:],
                            scale=1.0,
                            accum_out=sumcols[:, t : t + 1],
                        )
                        nc.scalar.activation(
                            out=sq_scratch[:, :tw],
                            in_=conv_sb[:, o : o + tw],
                            func=AF.Square,
                            accum_out=sqcols[:, t : t + 1],
                        )

            def bn_relu(lyr, act_out):
                """global BN stats allreduce + fused scale/shift/relu -> act_out fp16."""
                ssum = small.tile([F, 1], fp32, tag="ssum")
                nc.vector.tensor_reduce(
                    ssum[:], sumcols[:], axis=mybir.AxisListType.X, op=OP.add
                )
                ssq = small.tile([F, 1], fp32, tag="ssq")
                nc.vector.tensor_reduce(
                    ssq[:], sqcols[:], axis=mybir.AxisListType.X, op=OP.add
                )
                st = small.tile([F, 2], fp32, tag="stats")
                nc.vector.tensor_copy(st[:, 0:1], ssum[:])
                nc.vector.tensor_copy(st[:, 1:2], ssq[:])
                nc.sync.dma_start(out=stats_locs[lyr][:], in_=st[:])
                nc.gpsimd.collective_compute(
                    "AllReduce",
                    OP.add,
                    replica_groups=[list(range(N_CORES))],
                    ins=[stats_locs[lyr][:].opt()],
                    outs=[stats_globs[lyr][:].opt()],
                )
                stg = small.tile([F, 2], fp32, tag="statsg")
                nc.sync.dma_start(out=stg[:], in_=stats_globs[lyr][:])
                mean = small.tile([F, 1], fp32, tag="mean")
                nc.vector.tensor_scalar_mul(mean[:], stg[:, 0:1], 1.0 / N)
                ex2 = small.tile([F, 1], fp32, tag="ex2")
                nc.vector.tensor_scalar_mul(ex2[:], stg[:, 1:2], 1.0 / N)
                var = small.tile([F, 1], fp32, tag="var")
                nc.vector.tensor_tensor(var[:], mean[:], mean[:], op=OP.mult)
                nc.vector.tensor_sub(var[:], ex2[:], var[:])
                nc.vector.tensor_scalar_add(var[:], var[:], BN_EPS)
                std = small.tile([F, 1], fp32, tag="std")
                nc.scalar.sqrt(std[:], var[:])
                rstd = small.tile([F, 1], fp32, tag="rstd")
                nc.vector.reciprocal(rstd[:], std[:])
                scale = small.tile([F, 1], fp32, tag="scale")
                nc.vector.tensor_mul(scale[:], rstd[:], g_sb[lyr][:])
                shift = small.tile([F, 1], fp32, tag="shift")
                nc.vector.tensor_mul(shift[:], mean[:], scale[:])
                nc.vector.tensor_sub(shift[:], be_sb[lyr][:], shift[:])
                # chunked so downstream z-production starts on early chunks
                for (o, w) in zchunks:
                    nc.scalar.activation(
                        out=act_out[:, o : o + w],
                        in_=conv_sb[:, o : o + w],
                        func=AF.Relu,
                        bias=shift[:],
                        scale=scale[:],
                    )

            PHASE = int(os.environ.get("KGNN_PHASE", "9"))
            nc.vector.memset(actB[:], 0.0)
            nc.vector.memset(conv_sb[:], 0.0)
            # ---- layer 1 ----
            if PHASE >= 1:
                with nc.named_scope("z1"):
                    produce_z(actA, 0)
            if PHASE >= 2:
                with nc.named_scope("conv1"):
                    conv_layer(0)
            if PHASE >= 3:
                with nc.named_scope("bn1"):
                    bn_relu(0, actB)
            if PHASE >= 4:
                # ---- layer 2 ----
                with nc.named_scope("z2"):
                    produce_z(actB, 1)
                with nc.named_scope("conv2"):
                    conv_layer(1)
                with nc.named_scope("bn2"):
                    bn_relu(1, actA)
                    # jk12 = max(act1, act2) -> actB
                    nc.vector.tensor_max(actB[:], actB[:], actA[:])
            if PHASE >= 5:
                # ---- layer 3 ----
                with nc.named_scope("z3"):
                    produce_z(actA, 2)
                with nc.named_scope("conv3"):
                    conv_layer(2)
            # ---- JK max + projection, chunked to overlap conv3's tail ----
            for (o, w) in zchunks:
                # conv3 -> fp16, jk = max(jk12, conv3), project
                nc.scalar.copy(actA[:, o : o + w], conv_sb[:, o : o + w])
                nc.vector.tensor_max(
                    actB[:, o : o + w], actB[:, o : o + w], actA[:, o : o + w]
                )
                pp = z_ps.tile([F, ZCHUNK], fp32, tag="zps")
                nc.tensor.matmul(
                    pp[:OUTF, :w], lhsT=wp_sb[:], rhs=actB[:, o : o + w],
                    start=True, stop=True,
                )
                po = rstage.tile([OUTF, ZCHUNK], fp32, tag="pout")
                nc.scalar.activation(
                    out=po[:, :w], in_=pp[:OUTF, :w], func=AF.Identity,
                    bias=bp_sb[:], scale=1.0,
                )
                nc.sync.dma_start(out=out_ext[:, o : o + w], in_=po[:, :w])

    nc.compile()
    return nc


_CACHE = {}
_last_nc = None
_last_in_maps = None


def kernel(**inputs):
    global _last_nc, _last_in_maps
    from concourse.bass_utils import run_bass_kernel_spmd

    x = np.asarray(inputs["x"], dtype=np.float32)
    edge_index = np.asarray(inputs["edge_index"])

    ck = hash(edge_index.tobytes())
    if ck not in _CACHE:
        structure, per_core = _preprocess(edge_index)
        nc = _build(structure)
        _CACHE[ck] = (structure, per_core, nc)
    structure, per_core, nc = _CACHE[ck]

    in_maps = []
    for c in range(N_CORES):
        xc = x[c * SHARD : (c + 1) * SHARD].astype(np.float16)
        m = {
            "xT": np.ascontiguousarray(xc.T),
            "idx": per_core[c]["idx"],
            "smat": per_core[c]["smat"],
            "W1": np.asarray(inputs["W1"], np.float16),
            "W2": np.asarray(inputs["W2"], np.float16),
            "W3": np.asarray(inputs["W3"], np.float16),
            "Wp": np.asarray(inputs["Wp"], np.float16),
            "b1": np.asarray(inputs["b1"], np.float32).reshape(F, 1),
            "b2": np.asarray(inputs["b2"], np.float32).reshape(F, 1),
            "b3": np.asarray(inputs["b3"], np.float32).reshape(F, 1),
            "bp": np.asarray(inputs["bp"], np.float32).reshape(OUTF, 1),
            "g1": np.asarray(inputs["g1"], np.float32).reshape(F, 1),
            "g2": np.asarray(inputs["g2"], np.float32).reshape(F, 1),
            "be1": np.asarray(inputs["be1"], np.float32).reshape(F, 1),
            "be2": np.asarray(inputs["be2"], np.float32).reshape(F, 1),
        }
        in_maps.append(m)

    _last_nc, _last_in_maps = nc, in_maps
    res = run_bass_kernel_spmd(nc, in_maps, core_ids=list(range(N_CORES)))
    out = np.empty((N, OUTF), dtype=np.float32)
    for c in range(N_CORES):
        out[c * SHARD : (c + 1) * SHARD] = res.results[c]["outT"].T
    return out
